# revision 9
# baseline (speedup 1.0000x reference)
"""Self-contained trn2 Bass kernel: LSTM (B=64, T=512, I=128, H=1024,
forget_bias=1.0, tf gate order i,j,f,o) + per-timestep dense layer.

Strategy (8 NeuronCores, one chip): gate/hidden sharding of the recurrence.
Each core owns 128 hidden units. Per timestep it computes its 512 gate
columns for the full batch in transposed layout gates^T [128 gate-units,
64 batch] on the TensorEngine (Wh tiles stationary, h^T chunks moving,
PSUM accumulation; each of the 4 gate tiles f/i/o/j sits at the head of
its own PSUM bank so ScalarE can consume finished tiles while PE writes
later ones). The LSTM cell elementwise runs on ScalarE (sigmoid/tanh,
fp32) + VectorE; the new h^T chunk [128, 64] (bf16) is pushed SBUF->SBUF
to all 8 cores with remote_dma_broadcast. The dense layer is computed
every 16 steps from an SBUF history buffer, each core producing its own
8-row batch slice.

Perf notes (HW-measured): the per-step broadcast prep+trigger sequence has
a large fixed cost (~10us/step) that is independent of payload size,
descriptor count, and send pipelining -- it dominates the runtime. Two
mitigations are applied here: (1) sends are pipelined 4-deep (hstage is a
4-slot ring with per-slot completion semaphores; the trigger no longer
waits for the previous send's full round trip), and (2) the recurrent
weights for the three sigmoid gates (f, i, o) are stored fp8-e4m3 so their
LDWEIGHTS run at twice the bf16 rate, shrinking the PE block on the
critical path; the tanh candidate gate (j) weights stay bf16 for accuracy
(measured rel err 3.8e-3 vs 3.4e-3 all-bf16; all-fp8 would be 1.5e-2).
Matmuls accumulate fp32 in PSUM; cell state is fp32.
"""

from contextlib import ExitStack

import numpy as np
import ml_dtypes

import concourse.bass as bass
import concourse.bacc as bacc
import concourse.mybir as mybir
from concourse.alu_op_type import AluOpType
from concourse.bass_utils import run_bass_kernel_spmd

F32 = mybir.dt.float32
BF16 = mybir.dt.bfloat16
FP8 = mybir.dt.float8e4
AF = mybir.ActivationFunctionType

N_CORES = 8
B = 64
H = 1024
HC = H // N_CORES          # hidden units per core
G = 4 * HC                 # gate cols per core (tiles f,i,o,j)
PSUM_BANK = 512            # f32 elems per psum bank


def _build(T, dense_blk, include_bias, include_dense_bias):
    assert T % dense_blk == 0
    n_blk = T // dense_blk

    nc = bacc.Bacc(target_bir_lowering=False)

    xt_d = nc.declare_dram_parameter("XT", [128, T * B], BF16, isOutput=False)
    wx_d = nc.declare_dram_parameter("WX", [128, G], BF16, isOutput=False)
    wh8_d = nc.declare_dram_parameter("WH8", [H, 3 * HC], FP8, isOutput=False)
    whb_d = nc.declare_dram_parameter("WHB", [H, HC], BF16, isOutput=False)
    wd_d = nc.declare_dram_parameter("WD", [H, 128], BF16, isOutput=False)
    if include_bias:
        b_d = nc.declare_dram_parameter("B512", [1, G], BF16, isOutput=False)
    if include_dense_bias:
        bd_d = nc.declare_dram_parameter("BD", [1, 128], BF16, isOutput=False)
    out_d = nc.declare_dram_parameter("OUT", [T * 8, 128], F32, isOutput=True)

    XT_CHUNKS = 16 if T >= 64 else 1
    steps_per_chunk = T // XT_CHUNKS

    with ExitStack() as ctx:
        block = ctx.enter_context(nc.Block())
        sem = lambda n: ctx.enter_context(nc.semaphore(n))
        sb = lambda n, shape, dt: ctx.enter_context(nc.sbuf_tensor(n, shape, dt))
        wsem, xtsem, lsem, prep = sem("wsem"), sem("xtsem"), sem("lsem"), sem("prep")
        lsems = [sem(f"lsem{q}") for q in range(4)]
        hsems = [[sem(f"hsem{k}_{q}") for q in range(2)] for k in range(N_CORES)]

        def npar(q, j):
            # number of broadcasts among {-1, 0, .., j} with parity q
            return sum(1 for jj in range(-1, j + 1) if jj % 2 == q)

        gsem, asem, vsem, csem = sem("gsem"), sem("asem"), sem("vsem"), sem("csem")
        histsem, densesem, outcp, outdma, constsem = (
            sem("histsem"), sem("densesem"), sem("outcp"), sem("outdma"), sem("constsem"))
        xt_sb = sb("xt_sb", [128, T * B], BF16)
        wx_sb = sb("wx_sb", [128, G], BF16)
        wh8_sb = sb("wh8_sb", [128, 8 * 3 * HC], FP8)
        whb_sb = sb("whb_sb", [128, 8 * HC], BF16)
        wd_sb = sb("wd_sb", [128, 8 * 128], BF16)
        hbuf = sb("hbuf", [128, 2 * G], BF16)
        hstage = sb("hstage", [128, 4 * B], BF16)
        c_sb = sb("c_sb", [128, B], F32)
        sf_sb = sb("sf_sb", [128, B], F32)
        si_sb = sb("si_sb", [128, B], F32)
        so_sb = sb("so_sb", [128, B], F32)
        tj_sb = sb("tj_sb", [128, B], F32)
        tc_sb = sb("tc_sb", [128, B], F32)
        cf_sb = sb("cf_sb", [128, B], F32)
        t1_sb = sb("t1_sb", [128, B], F32)
        hist = sb("hist", [128, 2 * 8 * dense_blk * 8], BF16)
        ostage = sb("ostage", [128, 2 * 128], F32)
        gates_ps = ctx.enter_context(nc.psum_tensor("gates_ps", [128, 4 * PSUM_BANK], F32))
        dense_ps = ctx.enter_context(nc.psum_tensor("dense_ps", [128, 128], F32))
        if include_bias:
            ones_sb = sb("ones_sb", [1, B], BF16)
            b_sb = sb("b_sb", [1, G], BF16)
        if include_dense_bias:
            onesd_sb = sb("onesd_sb", [1, 128], BF16)
            bd_sb = sb("bd_sb", [1, 128], BF16)

        n_wdma = 4 + (1 if include_bias else 0) + (1 if include_dense_bias else 0)
        hist_cols = 8 * dense_blk * 8
        MD = dense_blk * 8

        def gtile(m):
            return gates_ps[:, m * PSUM_BANK : m * PSUM_BANK + B]


        @block.sync
        def _(s: bass.BassEngine):
            s.dma_start(out=wx_sb[:, :], in_=wx_d[:, :]).then_inc(wsem, 16)
            s.dma_start(
                out=wh8_sb[:, :].rearrange("p (c g) -> p c g", c=8),
                in_=wh8_d[:, :].rearrange("(c p) g -> p c g", p=128),
            ).then_inc(wsem, 16)
            s.dma_start(
                out=whb_sb[:, :].rearrange("p (c g) -> p c g", c=8),
                in_=whb_d[:, :].rearrange("(c p) g -> p c g", p=128),
            ).then_inc(wsem, 16)
            s.dma_start(
                out=wd_sb[:, :].rearrange("p (c o) -> p c o", c=8),
                in_=wd_d[:, :].rearrange("(c p) o -> p c o", p=128),
            ).then_inc(wsem, 16)
            if include_bias:
                s.dma_start(out=b_sb[:, :], in_=b_d[:, :]).then_inc(wsem, 16)
            if include_dense_bias:
                s.dma_start(out=bd_sb[:, :], in_=bd_d[:, :]).then_inc(wsem, 16)
            s.dma_start(out=xt_sb[:, :], in_=xt_d[:, :]).then_inc(xtsem, 16)
            for blk in range(n_blk):
                s.wait_ge(outcp, blk + 1)
                if blk >= 1:
                    s.wait_ge(outdma, 16 * blk)
                s.dma_start(
                    out=out_d[blk * MD : (blk + 1) * MD, :],
                    in_=ostage[:MD, (blk % 2) * 128 : (blk % 2) * 128 + 128],
                ).then_inc(outdma, 16)
            s.wait_ge(outdma, 16 * n_blk)

        n_consts = (1 if include_bias else 0) + (1 if include_dense_bias else 0)

        @block.tensor
        def _(e: bass.BassTensorEngine):
            e.wait_ge(wsem, 16 * n_wdma)
            if n_consts:
                e.wait_ge(constsem, n_consts)

            _ones_col_dense = onesd_sb[0:1, :] if include_dense_bias else None

            def dense_block(bi):
                bp = bi % 2
                e.wait_ge(histsem, dense_blk * (bi + 1))
                if bi >= 1:
                    e.wait_ge(outcp, bi)
                n_mm = 8 + (1 if include_dense_bias else 0)
                k = 0
                for c in range(8):
                    mm = e.matmul(
                        dense_ps[:MD, :],
                        lhsT=hist[:, bp * hist_cols + c * dense_blk * 8 :][
                            :, : dense_blk * 8
                        ],
                        rhs=wd_sb[:, c * 128 : (c + 1) * 128],
                        start=(k == 0),
                        stop=(k == n_mm - 1),
                        skip_group_check=True,
                    )
                    k += 1
                if include_dense_bias:
                    mm = e.matmul(
                        dense_ps[:MD, :],
                        lhsT=_ones_col_dense,
                        rhs=bd_sb[0:1, :],
                        start=False,
                        stop=True,
                        skip_group_check=True,
                    )
                mm.then_inc(densesem, 1)

            for t in range(T):
                pp = (t - 1) % 2
                if t == 0:
                    e.wait_ge(xtsem, 16)
                if t >= 1:
                    e.wait_ge(asem, 5 * (t - 1) + 4)
                for m in range(4):
                    mm = e.matmul(
                        gtile(m),
                        lhsT=wx_sb[:, m * 128 : (m + 1) * 128],
                        rhs=xt_sb[:, t * B : (t + 1) * B],
                        start=True,
                        stop=False,
                        skip_group_check=True,
                    )
                    if include_bias:
                        mm = e.matmul(
                            gtile(m),
                            lhsT=b_sb[0:1, m * 128 : (m + 1) * 128],
                            rhs=ones_sb[0:1, :],
                            start=False,
                            stop=False,
                            skip_group_check=True,
                        )
                if t >= 2 and (t - 2) % dense_blk == 0 and (t - 2) // dense_blk >= 1:
                    dense_block((t - 2) // dense_blk - 1)
                for k in range(N_CORES):
                    e.wait_ge(hsems[k][pp], 2 * npar(pp, t - 1))
                for m in range(4):
                    for c in range(8):
                        i8 = {0: 0, 1: 1, 3: 2}.get(m)
                        lhsT = (
                            wh8_sb[:, (c * 3 + i8) * 128 : (c * 3 + i8 + 1) * 128]
                            if i8 is not None
                            else whb_sb[:, c * 128 : (c + 1) * 128]
                        )
                        mm = e.matmul(
                            gtile(m),
                            lhsT=lhsT,
                            rhs=hbuf[:, pp * G + c * B : pp * G + (c + 1) * B],
                            start=False,
                            stop=(c == 7),
                            skip_group_check=True,
                        )
                    mm.then_inc(gsem, 1)
            dense_block(n_blk - 1)

        @block.scalar
        def _(a: bass.BassScalarEngine):
            for t in range(T):
                a.wait_ge(gsem, 4 * t + 1)
                if t >= 1:
                    a.wait_ge(vsem, 3 * (t - 1) + 2)
                a.activation(sf_sb[:, :], gtile(0), AF.Sigmoid, bias=1.0,
                             scale=1.0 / 64.0).then_inc(asem, 1)
                a.wait_ge(gsem, 4 * t + 2)
                if t >= 1:
                    a.wait_ge(vsem, 3 * (t - 1) + 3)
                a.activation(si_sb[:, :], gtile(1), AF.Sigmoid,
                             scale=1.0 / 64.0).then_inc(asem, 1)
                a.wait_ge(gsem, 4 * t + 3)
                if t >= 1:
                    a.wait_ge(vsem, 3 * (t - 1) + 3)
                a.activation(tj_sb[:, :], gtile(2), AF.Tanh,
                             scale=1.0 / 64.0).then_inc(asem, 1)
                a.wait_ge(gsem, 4 * t + 4)
                if t >= 1:
                    a.wait_ge(vsem, 3 * (t - 1) + 4)
                a.activation(so_sb[:, :], gtile(3), AF.Sigmoid,
                             scale=1.0 / 64.0).then_inc(asem, 1)
                a.wait_ge(csem, t + 2)
                if t >= 1:
                    a.wait_ge(vsem, 3 * (t - 1) + 4)
                a.activation(tc_sb[:, :], c_sb[:, :], AF.Tanh).then_inc(asem, 1)
                if t >= 3 and (t - 3) % dense_blk == 0 and (t - 3) // dense_blk >= 1:
                    bi = (t - 3) // dense_blk - 1
                    a.wait_ge(densesem, bi + 1)
                    if bi >= 2:
                        a.wait_ge(outdma, 16 * (bi - 1))
                    a.copy(
                        ostage[:MD, (bi % 2) * 128 : (bi % 2) * 128 + 128],
                        dense_ps[:MD, :],
                    ).then_inc(outcp, 1)
            bi = n_blk - 1
            a.wait_ge(densesem, bi + 1)
            if bi >= 1:
                a.wait_ge(outdma, 16 * (bi - 1))
            a.copy(
                ostage[:MD, (bi % 2) * 128 : (bi % 2) * 128 + 128], dense_ps[:MD, :]
            ).then_inc(outcp, 1)

        @block.vector
        def _(v: bass.BassVectorEngine):
            if include_bias:
                v.memset(ones_sb[:, :], 1.0).then_inc(constsem, 1)
            if include_dense_bias:
                v.memset(onesd_sb[:, :], 1.0).then_inc(constsem, 1)
            v.memset(c_sb[:, :], 0.0).then_inc(csem, 1)
            v.memset(hstage[:, 3 * B : 4 * B], 0.0).then_inc(vsem, 1)
            for t in range(T):
                # cf = sf * c
                v.wait_ge(asem, 5 * t + 1)
                v.wait_ge(csem, t + 1)
                v.tensor_tensor(
                    cf_sb[:, :], sf_sb[:, :], c_sb[:, :], AluOpType.mult
                ).then_inc(vsem, 1)
                # t1 = si * tj
                v.wait_ge(asem, 5 * t + 3)
                v.tensor_tensor(
                    t1_sb[:, :], si_sb[:, :], tj_sb[:, :], AluOpType.mult
                ).then_inc(vsem, 1)
                # c = cf + t1
                v.wait_ge(vsem, 3 * t + 3)
                if t >= 1:
                    v.wait_ge(asem, 5 * (t - 1) + 5)
                v.tensor_tensor(
                    c_sb[:, :], cf_sb[:, :], t1_sb[:, :], AluOpType.add
                ).then_inc(csem, 1)
                # h = so * tc
                v.wait_ge(asem, 5 * t + 5)
                if t >= 4:
                    q4 = t % 4
                    cnt4 = (t - 4 - q4) // 4 + 1 + (1 if q4 == 3 else 0)
                    v.wait_ge(lsems[q4], 16 * cnt4)
                elif t == 3:
                    v.wait_ge(lsems[3], 16)
                v.tensor_tensor(
                    hstage[:, (t % 4) * B : (t % 4 + 1) * B],
                    so_sb[:, :],
                    tc_sb[:, :],
                    AluOpType.mult,
                ).then_inc(vsem, 1)

        @block.gpsimd
        def _(g: bass.BassGpSimd):
            myg = g.partition_id()
            hb_r = hbuf[:, :].rearrange("p (q c v) -> p q c v", q=2, c=8)
            hist_r = hist[:, :].rearrange("p (q c w) -> p q c w", q=2, c=8)

            def hist_copy(tp, kk):
                ppp = tp % 2
                blk = tp // dense_blk
                tl = tp % dense_blk
                for k in range(N_CORES):
                    g.wait_ge(hsems[k][ppp], 2 * npar(ppp, tp))
                if tl == 0 and blk >= 2:
                    g.wait_ge(densesem, blk - 1)
                dst = hist_r[:, blk % 2, :, tl * 8 : tl * 8 + 8]
                src = hb_r[:, ppp, :, kk * 8 : kk * 8 + 8]
                g.tensor_copy(dst, src).then_inc(histsem, 1)
            for k in g.Switch(myg, N_CORES):
                g.remote_dma_broadcast(
                    out_ap=hbuf[:, G + k * B : G + (k + 1) * B],
                    in_ap=hstage[:, 3 * B : 4 * B],
                    remote_sem=hsems[k][1],
                    local_sem=lsems[3],
                    rdests=[(0, d) for d in range(N_CORES)],
                ).then_inc(prep, 1)
                g.wait_ge(prep, 1)
                g.wait_ge(vsem, 1)
                g.trigger_dma(count=1)
            for t in range(T):
                p = t % 2
                for k in g.Switch(myg, N_CORES):
                    g.remote_dma_broadcast(
                        out_ap=hbuf[:, p * G + k * B : p * G + (k + 1) * B],
                        in_ap=hstage[:, (t % 4) * B : (t % 4 + 1) * B],
                        remote_sem=hsems[k][p],
                        local_sem=lsems[t % 4],
                        rdests=[(0, d) for d in range(N_CORES)],
                    ).then_inc(prep, 1)
                    if t >= 1:
                        hist_copy(t - 1, k)
                        g.wait_ge(histsem, t)
                    g.wait_ge(prep, t + 2)
                    g.wait_ge(vsem, 3 * t + 4)
                    g.trigger_dma(count=1)
            for k in g.Switch(myg, N_CORES):
                hist_copy(T - 1, k)

    nc.finalize()
    return nc


_BUILD_CACHE = {}


def build(T, dense_blk, include_bias, include_dense_bias):
    return _build(T, dense_blk, include_bias, include_dense_bias)


def prep_inputs(X, Wx, Wh, b, Wd, bd):
    import ml_dtypes

    X = np.asarray(X, dtype=np.float32)
    Wx = np.asarray(Wx, dtype=np.float32)
    Wh = np.asarray(Wh, dtype=np.float32)
    b = np.asarray(b, dtype=np.float32)
    Wd = np.asarray(Wd, dtype=np.float32)
    bd = np.asarray(bd, dtype=np.float32)
    Bsz, T, _ = X.shape
    include_bias = bool(np.any(b))
    include_dense_bias = bool(np.any(bd))
    bf = ml_dtypes.bfloat16
    XT = np.ascontiguousarray(np.transpose(X, (2, 1, 0))).reshape(128, T * Bsz)
    in_maps = []
    for k in range(N_CORES):
        cols = []
        for gate in (2, 0, 1, 3):  # tiles f, i, j, o from reference order i,j,f,o
            lo = gate * H + k * HC
            cols.append(np.arange(lo, lo + HC))
        cols = np.concatenate(cols)
        f8 = ml_dtypes.float8_e4m3fn
        m = {
            "XT": XT.astype(bf),
            "WX": np.ascontiguousarray(64.0 * Wx[:, cols]).astype(bf),
            "WH8": np.ascontiguousarray(
                64.0 * Wh[:, np.r_[cols[:2 * HC], cols[3 * HC:]]]).astype(f8),
            "WHB": np.ascontiguousarray(
                64.0 * Wh[:, cols[2 * HC : 3 * HC]]).astype(bf),
            "WD": Wd.astype(bf),
        }
        if include_bias:
            m["B512"] = np.ascontiguousarray(64.0 * b[cols])[None, :].astype(bf)
        if include_dense_bias:
            m["BD"] = np.ascontiguousarray(bd)[None, :].astype(bf)
        in_maps.append(m)
    return in_maps


def assemble_output(results_list, T):
    outs = []
    for k in range(N_CORES):
        o = np.asarray(results_list[k]["OUT"]).reshape(T, 8, 128).transpose(1, 0, 2)
        outs.append(o)
    return np.concatenate(outs, axis=0).astype(np.float32)


def kernel(X, Wx, Wh, b, Wd, bd):
    X = np.asarray(X, dtype=np.float32)
    Bsz, T, _ = X.shape
    assert Bsz == B
    dense_blk = 16
    include_bias = bool(np.any(np.asarray(b)))
    include_dense_bias = bool(np.any(np.asarray(bd)))

    key = (T, dense_blk, include_bias, include_dense_bias)
    if key not in _BUILD_CACHE:
        _BUILD_CACHE[key] = _build(T, dense_blk, include_bias, include_dense_bias)
    nc = _BUILD_CACHE[key]

    in_maps = prep_inputs(X, Wx, Wh, b, Wd, bd)
    res = None
    for attempt in range(3):
        try:
            res = run_bass_kernel_spmd(nc, in_maps, core_ids=list(range(N_CORES)))
            break
        except Exception:
            if attempt == 2:
                raise
    return assemble_output([res.results[k] for k in range(N_CORES)], T=T)
